# revision 7
# baseline (speedup 1.0000x reference)
"""Trainium2 Bass kernel: per-point 3x3 Gaussian covariance from quaternion + log_scale.

cov = R diag(exp(log_scale)) R^T with R built from the normalized quaternion.

Strategy (v3, planar fp16):
  * Host reshapes inputs to struct-of-arrays fp16 planes per core:
    q [128, 4, R], ls [128, 3, R]; device writes the 6 unique entries of the
    symmetric cov as fp16 planes [128, 6, R]; host mirrors/casts to [N,3,3] f32.
  * Math: with half-square sums x0=(a^2+b^2-c^2-d^2)/2 etc. and unnormalized
    rotation half-columns x=(x0, bc+ad, bd-ac), y=(bc-ad, y1, cd+ab):
        cov = s2*I + alpha * x x^T + beta * y y^T
    where alpha=(s0-s2)*4/n^4, beta=(s1-s2)*4/n^4; the 4/n^4 comes via
    inv4 = exp(-2*ln(n^2/2)). Only TWO outer products thanks to
    sum_j r_j r_j^T = I.
  * All DVE ops are unit-stride fp16 [128, F] tiles -> 2x perf mode.
    ScalarE does squares/ln/exp; GpSimd owns the (2,2) gram entry and the
    diagonal +s2 adds as an independent chain.
"""

import os
import numpy as np

import concourse.bass as bass
import concourse.bacc as bacc
import concourse.mybir as mybir
from concourse.tile import TileContext
from concourse.bass_utils import run_bass_kernel_spmd

AF = mybir.ActivationFunctionType
OP = mybir.AluOpType
FP16 = mybir.dt.float16
FP32 = mybir.dt.float32

N_CORES = 8
N_FULL = 4_000_000
P = 128
R = 3912                      # rows per partition per core; 128*3912*8 >= 4M
NPC = P * R                   # points per core (padded)
F = int(os.environ.get("KERNEL_F", "1304"))   # main tile size (points/partition)
F0 = int(os.environ.get("KERNEL_F0", "489"))  # first/last (pipeline fill/drain) tile
GPS = int(os.environ.get("KERNEL_GPS", "0"))  # gpsimd offload level 0/1/2


def _tile_schedule():
    """Tile sizes summing to R: small first/last tiles to shrink fill/drain."""
    if F0 <= 0 or 2 * F0 >= R:
        sizes = []
        rem = R
        while rem > 0:
            fcur = min(F, rem)
            sizes.append(fcur)
            rem -= fcur
        return sizes
    mid = R - 2 * F0
    sizes = [F0]
    while mid > 0:
        fcur = min(F, mid)
        sizes.append(fcur)
        mid -= fcur
    sizes.append(F0)
    return sizes

SQRT_HALF = 0.7071067811865476

# output plane order: (i,k) pairs of the symmetric cov
PAIRS = [(0, 0), (0, 1), (0, 2), (1, 1), (1, 2), (2, 2)]

_built = {}


def _build():
    key = (F, GPS)
    if key in _built:
        return _built[key]

    nc = bacc.Bacc("TRN2", target_bir_lowering=False, debug=False, num_devices=N_CORES)
    q = nc.dram_tensor("q", [P, 4, R], FP16, kind="ExternalInput")
    ls = nc.dram_tensor("ls", [P, 3, R], FP16, kind="ExternalInput")
    cov = nc.dram_tensor("cov", [P, 6, R], FP16, kind="ExternalOutput")

    qv = q.ap()
    lsv = ls.ap()
    ov = cov.ap()

    with TileContext(nc) as tc:
        with (
            tc.tile_pool(name="io", bufs=2) as io,
            tc.tile_pool(name="otp", bufs=2) as otp,
            tc.tile_pool(name="wk2", bufs=2) as wk2,
            tc.tile_pool(name="wk1", bufs=1) as wk1,
        ):
            t0 = 0
            for f in _tile_schedule():
                _tile_body(nc, io, otp, wk2, wk1, qv, lsv, ov, t0, f)
                t0 += f

    nc.compile()
    _built[key] = nc
    return nc


def _tile_body(nc, io, otp, wk2, wk1, qv, lsv, ov, t0, f):
    def w2(tag):
        return wk2.tile([P, f], FP16, tag=tag, name=f"{tag}_{t0}")

    def w1(tag):
        return wk1.tile([P, f], FP16, tag=tag, name=f"{tag}_{t0}")

    V = nc.vector
    G = nc.gpsimd if GPS else nc.vector

    qt = io.tile([P, 4 * f], FP16, tag="qt", name=f"qt{t0}")
    lst = io.tile([P, 3 * f], FP16, tag="lst", name=f"lst{t0}")
    nc.sync.dma_start(out=qt.rearrange("p (c f) -> p c f", c=4), in_=qv[:, :, t0:t0 + f])
    nc.sync.dma_start(out=lst.rearrange("p (c f) -> p c f", c=3), in_=lsv[:, :, t0:t0 + f])

    a = qt[:, 0:f]; b = qt[:, f:2 * f]; c = qt[:, 2 * f:3 * f]; d = qt[:, 3 * f:4 * f]
    l0 = lst[:, 0:f]; l1 = lst[:, f:2 * f]; l2 = lst[:, 2 * f:3 * f]

    # --- scale exps (ScalarE, only needs lst) ----------------------------
    s0t = w2("s0t"); s1t = w2("s1t"); s2t = w2("s2t")
    nc.scalar.activation(s0t, l0, AF.Exp)
    nc.scalar.activation(s1t, l1, AF.Exp)
    nc.scalar.activation(s2t, l2, AF.Exp)

    # --- quaternion cross products (DVE, only needs qt) -------------------
    pbc = w1("pbc"); pad = w1("pad"); pbd = w1("pbd")
    pac = w1("pac"); pcd = w1("pcd"); pab = w1("pab")
    V.tensor_mul(pbc, b, c)
    V.tensor_mul(pad, a, d)
    V.tensor_mul(pbd, b, d)
    V.tensor_mul(pac, a, c)
    V.tensor_mul(pcd, c, d)
    V.tensor_mul(pab, a, b)
    x1 = w1("x1"); y0 = w1("y0"); x2 = w1("x2"); y2 = w1("y2")
    V.tensor_add(x1, pbc, pad)
    V.tensor_sub(y0, pbc, pad)
    V.tensor_sub(x2, pbd, pac)
    V.tensor_add(y2, pcd, pab)

    # --- squares (ScalarE): s* = (comp^2)/2 ------------------------------
    sa = w2("sa"); sb = w2("sb"); sc = w2("sc"); sd = w2("sd")
    nc.scalar.activation(sa, a, AF.Square, scale=SQRT_HALF)
    nc.scalar.activation(sb, b, AF.Square, scale=SQRT_HALF)
    nc.scalar.activation(sc, c, AF.Square, scale=SQRT_HALF)
    nc.scalar.activation(sd, d, AF.Square, scale=SQRT_HALF)

    # --- half-square combos (DVE) -----------------------------------------
    u = w1("u"); v = w1("v"); n2h = w2("n2h"); x0 = w1("x0")
    u2 = w1("u2"); v2 = w1("v2"); y1 = w1("y1")
    V.tensor_add(u, sa, sb)
    V.tensor_add(v, sc, sd)
    V.tensor_add(n2h, u, v)
    V.tensor_sub(x0, u, v)
    V.tensor_sub(u2, sa, sb)
    V.tensor_sub(v2, sc, sd)
    V.tensor_add(y1, u2, v2)

    # --- normalization: inv4 = 4/n^4 = exp(-2*ln(n^2/2)) (ScalarE chain) --
    ln = w2("ln"); inv4 = w2("inv4")
    nc.scalar.activation(ln, n2h, AF.Ln)
    nc.scalar.activation(inv4, ln, AF.Exp, scale=-2.0)

    # --- alpha/beta and weighted columns (DVE) ----------------------------
    # d0/d1/al/be reuse the u/v/u2/v2 buffers (dead after x0/y1)
    d0 = w1("u"); d1 = w1("v"); al = w1("u2"); be = w1("v2")
    V.tensor_sub(d0, s0t, s2t)
    V.tensor_sub(d1, s1t, s2t)
    V.tensor_mul(al, d0, inv4)
    V.tensor_mul(be, d1, inv4)
    X = [x0, x1, x2]
    Y = [y0, y1, y2]
    # W tiles reuse the product buffers (dead after the x/y combos)
    W0 = [w1("pbc"), w1("pad"), w1("pbd")]
    W1 = [w1("pac"), w1("pcd"), w1("pab")]
    # order: i=2 first so gpsimd's (2,2) chain can start early
    for i in (2, 0, 1):
        V.tensor_mul(W0[i], al, X[i])
        V.tensor_mul(W1[i], be, Y[i])

    # --- cov entries, written straight into the out tile ------------------
    # plane idx per PAIRS; (2,2) fully on gpsimd; diag +s2 on gpsimd
    ot = otp.tile([P, 6 * f], FP16, tag="ot", name=f"ot{t0}")

    def plane(idx):
        return ot[:, idx * f:(idx + 1) * f]

    g0 = w1("g0"); h0 = w1("h0"); g1 = w1("g1"); h1 = w1("h1")
    if GPS >= 2:
        g22 = w2("g22"); h22 = w2("h22"); q22 = w2("q22")
        G.tensor_mul(g22, W0[2], X[2])
        G.tensor_mul(h22, W1[2], Y[2])
        G.tensor_add(q22, g22, h22)
        G.tensor_add(plane(5), q22, s2t)
    else:
        q22 = w2("q22") if GPS else w1("q22")
        V.tensor_mul(g0, W0[2], X[2])
        V.tensor_mul(h0, W1[2], Y[2])
        V.tensor_add(q22, g0, h0)
        G.tensor_add(plane(5), q22, s2t)

    # diag (0,0) and (1,1): DVE muls+add, +s2 on G (gpsimd or DVE)
    dt0 = w2("dt0") if GPS else w1("dt0")
    dt1 = w2("dt1") if GPS else w1("dt1")
    V.tensor_mul(g0, W0[0], X[0])
    V.tensor_mul(h0, W1[0], Y[0])
    V.tensor_add(dt0, g0, h0)
    G.tensor_add(plane(0), dt0, s2t)
    V.tensor_mul(g1, W0[1], X[1])
    V.tensor_mul(h1, W1[1], Y[1])
    V.tensor_add(dt1, g1, h1)
    G.tensor_add(plane(3), dt1, s2t)

    # off-diagonals fully on DVE
    for idx, (i, k) in [(1, (0, 1)), (2, (0, 2)), (4, (1, 2))]:
        g, h = (g0, h0) if idx % 2 == 0 else (g1, h1)
        V.tensor_mul(g, W0[i], X[k])
        V.tensor_mul(h, W1[i], Y[k])
        V.tensor_add(plane(idx), g, h)

    nc.sync.dma_start(out=ov[:, :, t0:t0 + f], in_=ot.rearrange("p (c f) -> p c f", c=6))


def _pack_inputs(quaternion, log_scale):
    n = quaternion.shape[0]
    total = N_CORES * NPC
    qp = np.empty((total, 4), np.float16)
    lp = np.empty((total, 3), np.float16)
    qp[:n] = quaternion[:n]
    lp[:n] = log_scale[:n]
    if total > n:
        qp[n:] = np.array([1, 0, 0, 0], np.float16)
        lp[n:] = 0
    in_maps = []
    for i in range(N_CORES):
        sl = slice(i * NPC, (i + 1) * NPC)
        qc = np.ascontiguousarray(
            qp[sl].reshape(P, R, 4).transpose(0, 2, 1))
        lc = np.ascontiguousarray(
            lp[sl].reshape(P, R, 3).transpose(0, 2, 1))
        in_maps.append({"q": qc, "ls": lc})
    return in_maps


def _unpack_output(results, n):
    # device planes: [P, 6, R] fp16, order PAIRS
    planes = np.concatenate(
        [r["cov"].transpose(0, 2, 1).reshape(NPC, 6) for r in results], axis=0
    )[:n].astype(np.float32)
    out = np.empty((n, 3, 3), np.float32)
    out[:, 0, 0] = planes[:, 0]
    out[:, 0, 1] = planes[:, 1]; out[:, 1, 0] = planes[:, 1]
    out[:, 0, 2] = planes[:, 2]; out[:, 2, 0] = planes[:, 2]
    out[:, 1, 1] = planes[:, 3]
    out[:, 1, 2] = planes[:, 4]; out[:, 2, 1] = planes[:, 4]
    out[:, 2, 2] = planes[:, 5]
    return out


def kernel_with_stats(quaternion, log_scale, trace=False):
    quaternion = np.asarray(quaternion, dtype=np.float32)
    log_scale = np.asarray(log_scale, dtype=np.float32)
    n = quaternion.shape[0]
    nc = _build()
    in_maps = _pack_inputs(quaternion, log_scale)
    res = run_bass_kernel_spmd(nc, in_maps, core_ids=list(range(N_CORES)), trace=trace)
    out = _unpack_output(res.results, n)
    return out, res


def kernel(quaternion, log_scale):
    out, _ = kernel_with_stats(quaternion, log_scale, trace=False)
    return out


# revision 8
# speedup vs baseline: 1.0428x; 1.0428x over previous
"""Trainium2 Bass kernel: per-point 3x3 Gaussian covariance from quaternion + log_scale.

cov = R diag(exp(log_scale)) R^T with R built from the normalized quaternion.

Strategy (v5, planar fp16 + merged multi-AP instructions):
  * Host reshapes inputs to struct-of-arrays fp16 planes per core:
    q [128, 4, R], ls [128, 3, R]; device writes the 6 unique entries of the
    symmetric cov as fp16 planes [128, 6, R] (diag first); host mirrors/casts
    to [N,3,3] f32.
  * Math: with half-square sums x0=(a^2+b^2-c^2-d^2)/2 etc. and unnormalized
    rotation half-columns x=(x0, bc+ad, bd-ac), y=(bc-ad, y1, cd+ab):
        cov = s2*I + alpha * x x^T + beta * y y^T
    where alpha=(s0-s2)*4/n^4, beta=(s1-s2)*4/n^4 via inv4=exp(-2*ln(n^2/2)).
    Only TWO outer products thanks to sum_j r_j r_j^T = I.
  * Logical ops are packed into few wide DVE instructions using multi-dim
    access patterns (outer dims with arbitrary/zero/negative strides, unit
    inner stride keeps the 2x fp16 perf mode). 22 DVE + 4 ACT + 3 DMA
    instructions per tile.
"""

import os
import numpy as np

import concourse.bass as bass
import concourse.bacc as bacc
import concourse.mybir as mybir
from concourse.tile import TileContext
from concourse.bass_utils import run_bass_kernel_spmd

AF = mybir.ActivationFunctionType
OP = mybir.AluOpType
FP16 = mybir.dt.float16

N_CORES = 8
N_FULL = 4_000_000
P = 128
R = 3912                      # rows per partition per core; 128*3912*8 >= 4M
NPC = P * R                   # points per core (padded)
F = int(os.environ.get("KERNEL_F", "1152"))   # main tile size
F0 = int(os.environ.get("KERNEL_F0", "456"))  # fill/drain tile size

SQRT_HALF = 0.7071067811865476

_built = {}


def _tile_schedule():
    if F0 <= 0 or 2 * F0 >= R:
        sizes, rem = [], R
        while rem > 0:
            sizes.append(min(F, rem))
            rem -= sizes[-1]
        return sizes
    mid = R - F0 - F0
    sizes = [F0]
    while mid > 0:
        sizes.append(min(F, mid))
        mid -= sizes[-1]
    sizes.append(F0)
    return sizes


def _build():
    key = (F, F0)
    if key in _built:
        return _built[key]

    nc = bacc.Bacc("TRN2", target_bir_lowering=False, debug=False, num_devices=N_CORES)
    q = nc.dram_tensor("q", [P, 4, R], FP16, kind="ExternalInput")
    ls = nc.dram_tensor("ls", [P, 3, R], FP16, kind="ExternalInput")
    cov = nc.dram_tensor("cov", [P, 6, R], FP16, kind="ExternalOutput")

    qv, lsv, ov = q.ap(), ls.ap(), cov.ap()

    with TileContext(nc) as tc:
        with (
            tc.tile_pool(name="io", bufs=2) as io,
            tc.tile_pool(name="otp", bufs=2) as otp,
            tc.tile_pool(name="wk2", bufs=2) as wk2,
            tc.tile_pool(name="wk1", bufs=1) as wk1,
        ):
            t0 = 0
            for f in _tile_schedule():
                _tile_body(nc, io, otp, wk2, wk1, qv, lsv, ov, t0, f)
                t0 += f

    nc.compile()
    _built[key] = nc
    return nc


def _tile_body(nc, io, otp, wk2, wk1, qv, lsv, ov, t0, f):
    V = nc.vector

    def rows(ap, c):
        return ap.rearrange("p (c f) -> p c f", c=c)

    def bcast(ap_f, n):
        # [P, f] -> [P, n, f] with zero stride on the middle dim
        return ap_f.unsqueeze(1).broadcast_to([P, n, f])

    qt = io.tile([P, 4 * f], FP16, tag="qt", name=f"qt{t0}")
    lst = io.tile([P, 3 * f], FP16, tag="lst", name=f"lst{t0}")
    nc.sync.dma_start(out=rows(qt, 4), in_=qv[:, :, t0:t0 + f])
    nc.sync.dma_start(out=rows(lst, 3), in_=lsv[:, :, t0:t0 + f])
    qr = rows(qt, 4)  # (a, b, c, d)

    # ---- ScalarE: squares and scale exps (one instruction each) ----------
    sq4 = wk2.tile([P, 4 * f], FP16, tag="sq4", name=f"sq4_{t0}")   # sa sb sc sd
    sexp = wk2.tile([P, 3 * f], FP16, tag="sexp", name=f"sexp{t0}")  # s0 s1 s2
    nc.scalar.activation(sq4, qt, AF.Square, scale=SQRT_HALF)
    nc.scalar.activation(sexp, lst, AF.Exp)
    sr = rows(sq4, 4)

    # ---- cross products: prod6 = (ab, ac, ad, bc, bd, cd) ----------------
    prod6 = wk1.tile([P, 6 * f], FP16, tag="prod6", name=f"prod6_{t0}")
    pr = rows(prod6, 6)
    V.tensor_mul(pr[:, 0:3, :], bcast(qt[:, 0:f], 3), qr[:, 1:4, :])   # ab ac ad
    V.tensor_mul(pr[:, 3:5, :], bcast(qt[:, f:2 * f], 2), qr[:, 2:4, :])  # bc bd
    V.tensor_mul(pr[:, 5:6, :], qr[:, 2:3, :], qr[:, 3:4, :])          # cd

    # ---- half-square combos: uvz = (u, v, u2', v2) -----------------------
    uvz = wk1.tile([P, 6 * f], FP16, tag="uvz", name=f"uvz{t0}")  # sized 6f for m1 reuse
    ur = rows(uvz, 6)
    V.tensor_add(ur[:, 0:2, :], sr[:, 0:3:2, :], sr[:, 1:4:2, :])  # u=sa+sb, v=sc+sd
    V.tensor_sub(ur[:, 2:4, :], sr[:, 1:3, :], sr[:, 0:4:3, :])    # u2'=sb-sa, v2=sc-sd
    n2h = wk2.tile([P, f], FP16, tag="n2h", name=f"n2h{t0}")
    V.tensor_add(n2h, uvz[:, 0:f], uvz[:, f:2 * f])

    # ---- rotation half-columns: xy6 = (x0, x1, x2, y0, y1, y2) -----------
    xy6 = wk1.tile([P, 6 * f], FP16, tag="xy6", name=f"xy6_{t0}")
    xr = rows(xy6, 6)
    V.tensor_sub(xr[:, 0:5:4, :], ur[:, 0:4:3, :], ur[:, 1:3, :])  # x0=u-v, y1=v2-u2'
    V.tensor_sub(xr[:, 2:4, :], pr[:, 4:2:-1, :], pr[:, 1:3, :])   # x2=bd-ac, y0=bc-ad
    V.tensor_add(xr[:, 1:6:4, :], pr[:, 3:6:2, :], pr[:, 2::-2, :])  # x1=bc+ad, y2=cd+ab

    # ---- ScalarE: ln + inv4 ----------------------------------------------
    ln = wk2.tile([P, f], FP16, tag="ln", name=f"ln{t0}")
    inv4 = wk2.tile([P, f], FP16, tag="inv4", name=f"inv4_{t0}")
    nc.scalar.activation(ln, n2h, AF.Ln)
    nc.scalar.activation(inv4, ln, AF.Exp, scale=-2.0)

    # ---- alpha/beta ------------------------------------------------------
    dd = wk1.tile([P, 3 * f], FP16, tag="dd", name=f"dd{t0}")  # sized 3f for dtmp reuse
    V.tensor_sub(rows(dd[:, 0:2 * f], 2), rows(sexp[:, 0:2 * f], 2),
                 bcast(sexp[:, 2 * f:3 * f], 2))               # d0=s0-s2, d1=s1-s2
    ab2 = wk1.tile([P, 2 * f], FP16, tag="ab2", name=f"ab2_{t0}")
    V.tensor_mul(rows(ab2, 2), rows(dd[:, 0:2 * f], 2), bcast(inv4, 2))  # al, be

    # ---- weighted columns ------------------------------------------------
    w03 = wk1.tile([P, 3 * f], FP16, tag="w03", name=f"w03_{t0}")
    w13 = wk1.tile([P, 3 * f], FP16, tag="w13", name=f"w13_{t0}")
    V.tensor_mul(rows(w03, 3), bcast(ab2[:, 0:f], 3), xr[:, 0:3, :])
    V.tensor_mul(rows(w13, 3), bcast(ab2[:, f:2 * f], 3), xr[:, 3:6, :])

    # ---- gram entries: m = (m_00, m_11, m_22, m_01, m_02, m_12) ----------
    m0 = wk1.tile([P, 6 * f], FP16, tag="prod6", name=f"m0_{t0}")
    m1 = wk1.tile([P, 6 * f], FP16, tag="uvz", name=f"m1_{t0}")
    m0r, m1r = rows(m0, 6), rows(m1, 6)
    V.tensor_mul(m0r[:, 0:3, :], rows(w03, 3), xr[:, 0:3, :])          # diag
    V.tensor_mul(m0r[:, 3:5, :], bcast(w03[:, 0:f], 2), xr[:, 1:3, :])  # m01 m02
    V.tensor_mul(m0r[:, 5:6, :], rows(w03, 3)[:, 1:2, :], xr[:, 2:3, :])  # m12
    V.tensor_mul(m1r[:, 0:3, :], rows(w13, 3), xr[:, 3:6, :])
    V.tensor_mul(m1r[:, 3:5, :], bcast(w13[:, 0:f], 2), xr[:, 4:6, :])
    V.tensor_mul(m1r[:, 5:6, :], rows(w13, 3)[:, 1:2, :], xr[:, 5:6, :])

    # ---- cov planes (diag first) -----------------------------------------
    ot = otp.tile([P, 6 * f], FP16, tag="ot", name=f"ot{t0}")
    dtmp = wk1.tile([P, 3 * f], FP16, tag="dd", name=f"dtmp{t0}")
    V.tensor_add(rows(dtmp, 3), m0r[:, 0:3, :], m1r[:, 0:3, :])
    V.tensor_add(rows(ot[:, 0:3 * f], 3), rows(dtmp, 3),
                 bcast(sexp[:, 2 * f:3 * f], 3))                        # diag + s2
    V.tensor_add(rows(ot[:, 3 * f:6 * f], 3), m0r[:, 3:6, :], m1r[:, 3:6, :])

    nc.sync.dma_start(out=ov[:, :, t0:t0 + f], in_=rows(ot, 6))


def _pack_inputs(quaternion, log_scale):
    n = quaternion.shape[0]
    total = N_CORES * NPC
    qp = np.empty((total, 4), np.float16)
    lp = np.empty((total, 3), np.float16)
    qp[:n] = quaternion[:n]
    lp[:n] = log_scale[:n]
    if total > n:
        qp[n:] = np.array([1, 0, 0, 0], np.float16)
        lp[n:] = 0
    in_maps = []
    for i in range(N_CORES):
        sl = slice(i * NPC, (i + 1) * NPC)
        qc = np.ascontiguousarray(qp[sl].reshape(P, R, 4).transpose(0, 2, 1))
        lc = np.ascontiguousarray(lp[sl].reshape(P, R, 3).transpose(0, 2, 1))
        in_maps.append({"q": qc, "ls": lc})
    return in_maps


def _unpack_output(results, n):
    # device planes: [P, 6, R] fp16, order (c00, c11, c22, c01, c02, c12)
    planes = np.concatenate(
        [r["cov"].transpose(0, 2, 1).reshape(NPC, 6) for r in results], axis=0
    )[:n].astype(np.float32)
    out = np.empty((n, 3, 3), np.float32)
    out[:, 0, 0] = planes[:, 0]
    out[:, 1, 1] = planes[:, 1]
    out[:, 2, 2] = planes[:, 2]
    out[:, 0, 1] = planes[:, 3]; out[:, 1, 0] = planes[:, 3]
    out[:, 0, 2] = planes[:, 4]; out[:, 2, 0] = planes[:, 4]
    out[:, 1, 2] = planes[:, 5]; out[:, 2, 1] = planes[:, 5]
    return out


def kernel_with_stats(quaternion, log_scale, trace=False):
    quaternion = np.asarray(quaternion, dtype=np.float32)
    log_scale = np.asarray(log_scale, dtype=np.float32)
    n = quaternion.shape[0]
    nc = _build()
    in_maps = _pack_inputs(quaternion, log_scale)
    res = run_bass_kernel_spmd(nc, in_maps, core_ids=list(range(N_CORES)), trace=trace)
    out = _unpack_output(res.results, n)
    return out, res


def kernel(quaternion, log_scale):
    out, _ = kernel_with_stats(quaternion, log_scale, trace=False)
    return out


# revision 10
# speedup vs baseline: 1.0452x; 1.0023x over previous
"""Trainium2 Bass kernel: per-point 3x3 Gaussian covariance from quaternion + log_scale.

cov = R diag(exp(log_scale)) R^T with R built from the normalized quaternion.

Strategy (v5, planar fp16 + merged multi-AP instructions):
  * Host reshapes inputs to struct-of-arrays fp16 planes per core:
    q [128, 4, R], ls [128, 3, R]; device writes the 6 unique entries of the
    symmetric cov as fp16 planes [128, 6, R] (diag first); host mirrors/casts
    to [N,3,3] f32.
  * Math: with half-square sums x0=(a^2+b^2-c^2-d^2)/2 etc. and unnormalized
    rotation half-columns x=(x0, bc+ad, bd-ac), y=(bc-ad, y1, cd+ab):
        cov = s2*I + alpha * x x^T + beta * y y^T
    where alpha=(s0-s2)*4/n^4, beta=(s1-s2)*4/n^4 via inv4=exp(-2*ln(n^2/2)).
    Only TWO outer products thanks to sum_j r_j r_j^T = I.
  * Logical ops are packed into few wide DVE instructions using multi-dim
    access patterns (outer dims with arbitrary/zero/negative strides, unit
    inner stride keeps the 2x fp16 perf mode). 22 DVE + 4 ACT + 3 DMA
    instructions per tile.
"""

import os
import numpy as np

import concourse.bass as bass
import concourse.bacc as bacc
import concourse.mybir as mybir
from concourse.tile import TileContext
from concourse.bass_utils import run_bass_kernel_spmd

AF = mybir.ActivationFunctionType
OP = mybir.AluOpType
FP16 = mybir.dt.float16
FP32 = mybir.dt.float32

N_CORES = 8
N_FULL = 4_000_000
P = 128
R = 3912                      # rows per partition per core; 128*3912*8 >= 4M
NPC = P * R                   # points per core (padded)
F = int(os.environ.get("KERNEL_F", "1152"))   # main tile size
F0 = int(os.environ.get("KERNEL_F0", "456"))  # fill/drain tile size

SQRT_HALF = 0.7071067811865476

_built = {}


def _tile_schedule():
    if F0 <= 0 or 2 * F0 >= R:
        sizes, rem = [], R
        while rem > 0:
            sizes.append(min(F, rem))
            rem -= sizes[-1]
        return sizes
    mid = R - F0 - F0
    sizes = [F0]
    while mid > 0:
        sizes.append(min(F, mid))
        mid -= sizes[-1]
    sizes.append(F0)
    return sizes


def _build():
    key = (F, F0)
    if key in _built:
        return _built[key]

    nc = bacc.Bacc("TRN2", target_bir_lowering=False, debug=False, num_devices=N_CORES)
    q = nc.dram_tensor("q", [P, 4, R], FP16, kind="ExternalInput")
    ls = nc.dram_tensor("ls", [P, 3, R], FP16, kind="ExternalInput")
    cov = nc.dram_tensor("cov", [P, 6, R], FP16, kind="ExternalOutput")

    qv, lsv, ov = q.ap(), ls.ap(), cov.ap()

    with TileContext(nc) as tc:
        with (
            tc.tile_pool(name="io", bufs=2) as io,
            tc.tile_pool(name="otp", bufs=2) as otp,
            tc.tile_pool(name="wk2", bufs=2) as wk2,
            tc.tile_pool(name="wk1", bufs=1) as wk1,
        ):
            t0 = 0
            for f in _tile_schedule():
                _tile_body(nc, io, otp, wk2, wk1, qv, lsv, ov, t0, f)
                t0 += f

    nc.compile()
    _built[key] = nc
    return nc


def _tile_body(nc, io, otp, wk2, wk1, qv, lsv, ov, t0, f):
    V = nc.vector

    def rows(ap, c):
        return ap.rearrange("p (c f) -> p c f", c=c)

    def bcast(ap_f, n):
        # [P, f] -> [P, n, f] with zero stride on the middle dim
        return ap_f.unsqueeze(1).broadcast_to([P, n, f])

    qt = io.tile([P, 4 * f], FP16, tag="qt", name=f"qt{t0}")
    lst = io.tile([P, 3 * f], FP16, tag="lst", name=f"lst{t0}")
    nc.sync.dma_start(out=rows(qt, 4), in_=qv[:, :, t0:t0 + f])
    nc.sync.dma_start(out=rows(lst, 3), in_=lsv[:, :, t0:t0 + f])
    qr = rows(qt, 4)  # (a, b, c, d)

    # ---- ScalarE: squares and scale exps (one instruction each) ----------
    sq4 = wk2.tile([P, 4 * f], FP16, tag="sq4", name=f"sq4_{t0}")   # sa sb sc sd
    sexp = wk2.tile([P, 3 * f], FP16, tag="sexp", name=f"sexp{t0}")  # s0 s1 s2
    nc.scalar.activation(sq4, qt, AF.Square, scale=SQRT_HALF)
    nc.scalar.activation(sexp, lst, AF.Exp)
    sr = rows(sq4, 4)

    # ---- cross products: prod6 = (ab, ac, ad, bc, bd, cd) ----------------
    prod6 = wk1.tile([P, 6 * f], FP16, tag="prod6", name=f"prod6_{t0}")
    pr = rows(prod6, 6)
    V.tensor_mul(pr[:, 0:3, :], bcast(qt[:, 0:f], 3), qr[:, 1:4, :])   # ab ac ad
    V.tensor_mul(pr[:, 3:5, :], bcast(qt[:, f:2 * f], 2), qr[:, 2:4, :])  # bc bd
    V.tensor_mul(pr[:, 5:6, :], qr[:, 2:3, :], qr[:, 3:4, :])          # cd

    # ---- half-square combos: uvz = (u, v, u2', v2) -----------------------
    uvz = wk1.tile([P, 6 * f], FP16, tag="uvz", name=f"uvz{t0}")  # sized 6f for m1 reuse
    ur = rows(uvz, 6)
    V.tensor_add(ur[:, 0:2, :], sr[:, 0:3:2, :], sr[:, 1:4:2, :])  # u=sa+sb, v=sc+sd
    V.tensor_sub(ur[:, 2:4, :], sr[:, 1:3, :], sr[:, 0:4:3, :])    # u2'=sb-sa, v2=sc-sd
    n2h = wk2.tile([P, f], FP16, tag="n2h", name=f"n2h{t0}")
    V.tensor_add(n2h, uvz[:, 0:f], uvz[:, f:2 * f])

    # ---- rotation half-columns: xy6 = (x0, x1, x2, y0, y1, y2) -----------
    xy6 = wk1.tile([P, 6 * f], FP16, tag="xy6", name=f"xy6_{t0}")
    xr = rows(xy6, 6)
    V.tensor_sub(xr[:, 0:5:4, :], ur[:, 0:4:3, :], ur[:, 1:3, :])  # x0=u-v, y1=v2-u2'
    V.tensor_sub(xr[:, 2:4, :], pr[:, 4:2:-1, :], pr[:, 1:3, :])   # x2=bd-ac, y0=bc-ad
    V.tensor_add(xr[:, 1:6:4, :], pr[:, 3:6:2, :], pr[:, 2::-2, :])  # x1=bc+ad, y2=cd+ab

    # ---- inv4 = 4/n^4 = 1/(n2h^2), avoiding the Ln table-set swap --------
    # ACT Square (same table set as Exp) -> fp32; DVE fast reciprocal (fp32);
    # ACT copy-cast back to fp16 so downstream ops keep the 2x perf mode.
    n4h = wk2.tile([P, f], FP32, tag="n4h", name=f"n4h{t0}")
    inv4f = wk2.tile([P, f], FP32, tag="inv4f", name=f"inv4f_{t0}")
    inv4 = wk2.tile([P, f], FP16, tag="inv4", name=f"inv4_{t0}")
    nc.scalar.activation(n4h, n2h, AF.Square)
    V.reciprocal_approx_fast(inv4f, n4h)
    nc.scalar.copy(inv4, inv4f)

    # ---- alpha/beta ------------------------------------------------------
    dd = wk1.tile([P, 3 * f], FP16, tag="dd", name=f"dd{t0}")  # sized 3f for dtmp reuse
    V.tensor_sub(rows(dd[:, 0:2 * f], 2), rows(sexp[:, 0:2 * f], 2),
                 bcast(sexp[:, 2 * f:3 * f], 2))               # d0=s0-s2, d1=s1-s2
    ab2 = wk1.tile([P, 2 * f], FP16, tag="ab2", name=f"ab2_{t0}")
    V.tensor_mul(rows(ab2, 2), rows(dd[:, 0:2 * f], 2), bcast(inv4, 2))  # al, be

    # ---- weighted columns ------------------------------------------------
    w03 = wk1.tile([P, 3 * f], FP16, tag="w03", name=f"w03_{t0}")
    w13 = wk1.tile([P, 3 * f], FP16, tag="w13", name=f"w13_{t0}")
    V.tensor_mul(rows(w03, 3), bcast(ab2[:, 0:f], 3), xr[:, 0:3, :])
    V.tensor_mul(rows(w13, 3), bcast(ab2[:, f:2 * f], 3), xr[:, 3:6, :])

    # ---- gram entries: m = (m_00, m_11, m_22, m_01, m_02, m_12) ----------
    m0 = wk1.tile([P, 6 * f], FP16, tag="prod6", name=f"m0_{t0}")
    m1 = wk1.tile([P, 6 * f], FP16, tag="uvz", name=f"m1_{t0}")
    m0r, m1r = rows(m0, 6), rows(m1, 6)
    V.tensor_mul(m0r[:, 0:3, :], rows(w03, 3), xr[:, 0:3, :])          # diag
    V.tensor_mul(m0r[:, 3:5, :], bcast(w03[:, 0:f], 2), xr[:, 1:3, :])  # m01 m02
    V.tensor_mul(m0r[:, 5:6, :], rows(w03, 3)[:, 1:2, :], xr[:, 2:3, :])  # m12
    V.tensor_mul(m1r[:, 0:3, :], rows(w13, 3), xr[:, 3:6, :])
    V.tensor_mul(m1r[:, 3:5, :], bcast(w13[:, 0:f], 2), xr[:, 4:6, :])
    V.tensor_mul(m1r[:, 5:6, :], rows(w13, 3)[:, 1:2, :], xr[:, 5:6, :])

    # ---- cov planes (diag first) -----------------------------------------
    ot = otp.tile([P, 6 * f], FP16, tag="ot", name=f"ot{t0}")
    dtmp = wk1.tile([P, 3 * f], FP16, tag="dd", name=f"dtmp{t0}")
    V.tensor_add(rows(dtmp, 3), m0r[:, 0:3, :], m1r[:, 0:3, :])
    V.tensor_add(rows(ot[:, 0:3 * f], 3), rows(dtmp, 3),
                 bcast(sexp[:, 2 * f:3 * f], 3))                        # diag + s2
    V.tensor_add(rows(ot[:, 3 * f:6 * f], 3), m0r[:, 3:6, :], m1r[:, 3:6, :])

    nc.sync.dma_start(out=ov[:, :, t0:t0 + f], in_=rows(ot, 6))


def _pack_inputs(quaternion, log_scale):
    n = quaternion.shape[0]
    total = N_CORES * NPC
    qp = np.empty((total, 4), np.float16)
    lp = np.empty((total, 3), np.float16)
    qp[:n] = quaternion[:n]
    lp[:n] = log_scale[:n]
    if total > n:
        qp[n:] = np.array([1, 0, 0, 0], np.float16)
        lp[n:] = 0
    in_maps = []
    for i in range(N_CORES):
        sl = slice(i * NPC, (i + 1) * NPC)
        qc = np.ascontiguousarray(qp[sl].reshape(P, R, 4).transpose(0, 2, 1))
        lc = np.ascontiguousarray(lp[sl].reshape(P, R, 3).transpose(0, 2, 1))
        in_maps.append({"q": qc, "ls": lc})
    return in_maps


def _unpack_output(results, n):
    # device planes: [P, 6, R] fp16, order (c00, c11, c22, c01, c02, c12)
    planes = np.concatenate(
        [r["cov"].transpose(0, 2, 1).reshape(NPC, 6) for r in results], axis=0
    )[:n].astype(np.float32)
    out = np.empty((n, 3, 3), np.float32)
    out[:, 0, 0] = planes[:, 0]
    out[:, 1, 1] = planes[:, 1]
    out[:, 2, 2] = planes[:, 2]
    out[:, 0, 1] = planes[:, 3]; out[:, 1, 0] = planes[:, 3]
    out[:, 0, 2] = planes[:, 4]; out[:, 2, 0] = planes[:, 4]
    out[:, 1, 2] = planes[:, 5]; out[:, 2, 1] = planes[:, 5]
    return out


def kernel_with_stats(quaternion, log_scale, trace=False):
    quaternion = np.asarray(quaternion, dtype=np.float32)
    log_scale = np.asarray(log_scale, dtype=np.float32)
    n = quaternion.shape[0]
    nc = _build()
    in_maps = _pack_inputs(quaternion, log_scale)
    res = run_bass_kernel_spmd(nc, in_maps, core_ids=list(range(N_CORES)), trace=trace)
    out = _unpack_output(res.results, n)
    return out, res


def kernel(quaternion, log_scale):
    out, _ = kernel_with_stats(quaternion, log_scale, trace=False)
    return out


# revision 13
# speedup vs baseline: 1.1232x; 1.0747x over previous
"""Trainium2 Bass kernel: per-point 3x3 Gaussian covariance from quaternion + log_scale.

cov = R diag(exp(log_scale)) R^T with R built from the normalized quaternion.

Strategy (v5, planar fp16 + merged multi-AP instructions):
  * Host reshapes inputs to struct-of-arrays fp16 planes per core:
    q [128, 4, R], ls [128, 3, R]; device writes the 6 unique entries of the
    symmetric cov as fp16 planes [128, 6, R] (diag first); host mirrors/casts
    to [N,3,3] f32.
  * Math: with half-square sums x0=(a^2+b^2-c^2-d^2)/2 etc. and unnormalized
    rotation half-columns x=(x0, bc+ad, bd-ac), y=(bc-ad, y1, cd+ab):
        cov = s2*I + alpha * x x^T + beta * y y^T
    where alpha=(s0-s2)*4/n^4, beta=(s1-s2)*4/n^4 via inv4=exp(-2*ln(n^2/2)).
    Only TWO outer products thanks to sum_j r_j r_j^T = I.
  * Logical ops are packed into few wide DVE instructions using multi-dim
    access patterns (outer dims with arbitrary/zero/negative strides, unit
    inner stride keeps the 2x fp16 perf mode). 22 DVE + 4 ACT + 3 DMA
    instructions per tile.
"""

import os
import numpy as np

import concourse.bass as bass
import concourse.bacc as bacc
import concourse.mybir as mybir
from concourse.tile import TileContext
from concourse.bass_utils import run_bass_kernel_spmd

AF = mybir.ActivationFunctionType
OP = mybir.AluOpType
FP16 = mybir.dt.float16
FP32 = mybir.dt.float32

N_CORES = 8
N_FULL = 4_000_000
P = 128
R = 3912                      # rows per partition per core; 128*3912*8 >= 4M
NPC = P * R                   # points per core (padded)
F = int(os.environ.get("KERNEL_F", "1184"))   # main tile size
F0 = int(os.environ.get("KERNEL_F0", "360"))  # fill tile size

SQRT_HALF = 0.7071067811865476

_built = {}


def _tile_schedule():
    """Small first tile to shrink pipeline fill, then even F-sized tiles."""
    sizes, rem = ([F0], R - F0) if 0 < F0 < R else ([], R)
    while rem > 0:
        fcur = min(F, rem)
        if fcur % 2:
            fcur += 1 if rem > fcur else -1
        sizes.append(min(fcur, rem))
        rem -= sizes[-1]
    return sizes


def _build():
    key = (F, F0)
    if key in _built:
        return _built[key]

    nc = bacc.Bacc("TRN2", target_bir_lowering=False, debug=False, num_devices=N_CORES)
    q = nc.dram_tensor("q", [P, 4, R], FP16, kind="ExternalInput")
    ls = nc.dram_tensor("ls", [P, 3, R], FP16, kind="ExternalInput")
    cov = nc.dram_tensor("cov", [P, 6, R], FP16, kind="ExternalOutput")

    qv, lsv, ov = q.ap(), ls.ap(), cov.ap()

    with TileContext(nc) as tc:
        with (
            tc.tile_pool(name="io", bufs=2) as io,
            tc.tile_pool(name="otp", bufs=2) as otp,
            tc.tile_pool(name="wk2", bufs=2) as wk2,
            tc.tile_pool(name="wk1", bufs=1) as wk1,
        ):
            t0 = 0
            for f in _tile_schedule():
                _tile_body(nc, io, otp, wk2, wk1, qv, lsv, ov, t0, f)
                t0 += f

    nc.compile()
    _built[key] = nc
    return nc


def _tile_body(nc, io, otp, wk2, wk1, qv, lsv, ov, t0, f):
    V = nc.vector

    def rows(ap, c):
        return ap.rearrange("p (c f) -> p c f", c=c)

    def bcast(ap_f, n):
        # [P, f] -> [P, n, f] with zero stride on the middle dim
        return ap_f.unsqueeze(1).broadcast_to([P, n, f])

    qt = io.tile([P, 4 * f], FP16, tag="qt", name=f"qt{t0}")
    lst = io.tile([P, 3 * f], FP16, tag="lst", name=f"lst{t0}")
    nc.sync.dma_start(out=rows(qt, 4), in_=qv[:, :, t0:t0 + f])
    nc.sync.dma_start(out=rows(lst, 3), in_=lsv[:, :, t0:t0 + f])
    qr = rows(qt, 4)  # (a, b, c, d)

    # ---- ScalarE: squares and scale exps (one instruction each) ----------
    sq4 = wk2.tile([P, 4 * f], FP16, tag="sq4", name=f"sq4_{t0}")   # sa sb sc sd
    sexp = wk2.tile([P, 3 * f], FP16, tag="sexp", name=f"sexp{t0}")  # s0 s1 s2
    nc.scalar.activation(sq4, qt, AF.Square, scale=SQRT_HALF)
    nc.scalar.activation(sexp, lst, AF.Exp)
    sr = rows(sq4, 4)

    # ---- cross products: prod6 = (ab, ac, ad, bc, bd, cd) ----------------
    prod6 = wk1.tile([P, 6 * f], FP16, tag="prod6", name=f"prod6_{t0}")
    pr = rows(prod6, 6)
    V.tensor_mul(pr[:, 0:3, :], bcast(qt[:, 0:f], 3), qr[:, 1:4, :])   # ab ac ad
    V.tensor_mul(pr[:, 3:5, :], bcast(qt[:, f:2 * f], 2), qr[:, 2:4, :])  # bc bd
    V.tensor_mul(pr[:, 5:6, :], qr[:, 2:3, :], qr[:, 3:4, :])          # cd

    # ---- half-square combos: uvz = (u, v, u2', v2) -----------------------
    uvz = wk1.tile([P, 6 * f], FP16, tag="uvz", name=f"uvz{t0}")  # sized 6f for m1 reuse
    ur = rows(uvz, 6)
    V.tensor_add(ur[:, 0:2, :], sr[:, 0:3:2, :], sr[:, 1:4:2, :])  # u=sa+sb, v=sc+sd
    V.tensor_sub(ur[:, 2:4, :], sr[:, 1:3, :], sr[:, 0:4:3, :])    # u2'=sb-sa, v2=sc-sd
    n2h = wk2.tile([P, f], FP16, tag="n2h", name=f"n2h{t0}")
    V.tensor_add(n2h, uvz[:, 0:f], uvz[:, f:2 * f])

    # ---- rotation half-columns: xy6 = (x0, x1, x2, y0, y1, y2) -----------
    xy6 = wk1.tile([P, 6 * f], FP16, tag="xy6", name=f"xy6_{t0}")
    xr = rows(xy6, 6)
    V.tensor_sub(xr[:, 0:5:4, :], ur[:, 0:4:3, :], ur[:, 1:3, :])  # x0=u-v, y1=v2-u2'
    V.tensor_sub(xr[:, 2:4, :], pr[:, 4:2:-1, :], pr[:, 1:3, :])   # x2=bd-ac, y0=bc-ad
    V.tensor_add(xr[:, 1:6:4, :], pr[:, 3:6:2, :], pr[:, 2::-2, :])  # x1=bc+ad, y2=cd+ab

    # ---- inv4 = 4/n^4 = 1/(n2h^2), avoiding the Ln table-set swap --------
    # ACT Square (same table set as Exp), then the fast-reciprocal custom DVE
    # op directly in fp16 (the BITWISE_NOT seed acts on the pipe's internal
    # fp32 conversion, so 16-bit I/O is fine and keeps downstream ops at 2x).
    from concourse.dve_ops import RECIP_APPROX_FAST_CONSTS, RECIPROCAL_APPROX_FAST
    n4h = wk2.tile([P, f], FP16, tag="n4h", name=f"n4h{t0}")
    inv4 = wk2.tile([P, f], FP16, tag="inv4", name=f"inv4_{t0}")
    nc.scalar.activation(n4h, n2h, AF.Square)
    rc = RECIP_APPROX_FAST_CONSTS
    V._custom_dve(RECIPROCAL_APPROX_FAST, out=inv4, in0=n4h,
                  s0=rc["s0"], s1=rc["s1"], imm2=rc["imm2"])

    # ---- alpha/beta ------------------------------------------------------
    dd = wk1.tile([P, 3 * f], FP16, tag="dd", name=f"dd{t0}")  # sized 3f for dtmp reuse
    V.tensor_sub(rows(dd[:, 0:2 * f], 2), rows(sexp[:, 0:2 * f], 2),
                 bcast(sexp[:, 2 * f:3 * f], 2))               # d0=s0-s2, d1=s1-s2
    ab2 = wk1.tile([P, 2 * f], FP16, tag="ab2", name=f"ab2_{t0}")
    V.tensor_mul(rows(ab2, 2), rows(dd[:, 0:2 * f], 2), bcast(inv4, 2))  # al, be

    # ---- weighted columns ------------------------------------------------
    w03 = wk1.tile([P, 3 * f], FP16, tag="w03", name=f"w03_{t0}")
    w13 = wk1.tile([P, 3 * f], FP16, tag="w13", name=f"w13_{t0}")
    V.tensor_mul(rows(w03, 3), bcast(ab2[:, 0:f], 3), xr[:, 0:3, :])
    V.tensor_mul(rows(w13, 3), bcast(ab2[:, f:2 * f], 3), xr[:, 3:6, :])

    # ---- gram entries: m = (m_00, m_11, m_22, m_01, m_02, m_12) ----------
    m0 = wk1.tile([P, 6 * f], FP16, tag="prod6", name=f"m0_{t0}")
    m1 = wk1.tile([P, 6 * f], FP16, tag="uvz", name=f"m1_{t0}")
    m0r, m1r = rows(m0, 6), rows(m1, 6)
    V.tensor_mul(m0r[:, 0:3, :], rows(w03, 3), xr[:, 0:3, :])          # diag
    V.tensor_mul(m0r[:, 3:5, :], bcast(w03[:, 0:f], 2), xr[:, 1:3, :])  # m01 m02
    V.tensor_mul(m0r[:, 5:6, :], rows(w03, 3)[:, 1:2, :], xr[:, 2:3, :])  # m12
    V.tensor_mul(m1r[:, 0:3, :], rows(w13, 3), xr[:, 3:6, :])
    V.tensor_mul(m1r[:, 3:5, :], bcast(w13[:, 0:f], 2), xr[:, 4:6, :])
    V.tensor_mul(m1r[:, 5:6, :], rows(w13, 3)[:, 1:2, :], xr[:, 5:6, :])

    # ---- cov planes (diag first) -----------------------------------------
    ot = otp.tile([P, 6 * f], FP16, tag="ot", name=f"ot{t0}")
    dtmp = wk1.tile([P, 3 * f], FP16, tag="dd", name=f"dtmp{t0}")
    V.tensor_add(rows(dtmp, 3), m0r[:, 0:3, :], m1r[:, 0:3, :])
    V.tensor_add(rows(ot[:, 0:3 * f], 3), rows(dtmp, 3),
                 bcast(sexp[:, 2 * f:3 * f], 3))                        # diag + s2
    V.tensor_add(rows(ot[:, 3 * f:6 * f], 3), m0r[:, 3:6, :], m1r[:, 3:6, :])

    nc.sync.dma_start(out=ov[:, :, t0:t0 + f], in_=rows(ot, 6))


def _pack_inputs(quaternion, log_scale):
    n = quaternion.shape[0]
    total = N_CORES * NPC
    qp = np.empty((total, 4), np.float16)
    lp = np.empty((total, 3), np.float16)
    qp[:n] = quaternion[:n]
    lp[:n] = log_scale[:n]
    if total > n:
        qp[n:] = np.array([1, 0, 0, 0], np.float16)
        lp[n:] = 0
    in_maps = []
    for i in range(N_CORES):
        sl = slice(i * NPC, (i + 1) * NPC)
        qc = np.ascontiguousarray(qp[sl].reshape(P, R, 4).transpose(0, 2, 1))
        lc = np.ascontiguousarray(lp[sl].reshape(P, R, 3).transpose(0, 2, 1))
        in_maps.append({"q": qc, "ls": lc})
    return in_maps


def _unpack_output(results, n):
    # device planes: [P, 6, R] fp16, order (c00, c11, c22, c01, c02, c12)
    planes = np.concatenate(
        [r["cov"].transpose(0, 2, 1).reshape(NPC, 6) for r in results], axis=0
    )[:n].astype(np.float32)
    out = np.empty((n, 3, 3), np.float32)
    out[:, 0, 0] = planes[:, 0]
    out[:, 1, 1] = planes[:, 1]
    out[:, 2, 2] = planes[:, 2]
    out[:, 0, 1] = planes[:, 3]; out[:, 1, 0] = planes[:, 3]
    out[:, 0, 2] = planes[:, 4]; out[:, 2, 0] = planes[:, 4]
    out[:, 1, 2] = planes[:, 5]; out[:, 2, 1] = planes[:, 5]
    return out


def kernel_with_stats(quaternion, log_scale, trace=False):
    quaternion = np.asarray(quaternion, dtype=np.float32)
    log_scale = np.asarray(log_scale, dtype=np.float32)
    n = quaternion.shape[0]
    nc = _build()
    in_maps = _pack_inputs(quaternion, log_scale)
    res = run_bass_kernel_spmd(nc, in_maps, core_ids=list(range(N_CORES)), trace=trace)
    out = _unpack_output(res.results, n)
    return out, res


def kernel(quaternion, log_scale):
    out, _ = kernel_with_stats(quaternion, log_scale, trace=False)
    return out
